# revision 1
# baseline (speedup 1.0000x reference)
"""ChebNet (3x ChebConv S=5 + global mean pool + 2-layer MLP) on 8 trn2 cores.

Strategy (graph-level data parallelism):
  - 64 independent graphs (1024 nodes each, edges strictly intra-graph).
    Core c owns graphs [8c, 8c+8).
  - Host prep: build per-graph dense scaled Laplacian  M2[src, dst] =
    2*Lhat[dst, src]  (edge weights + diagonal folded in, duplicate edges
    accumulated), transpose x per graph to feature-layout, concat the 5
    Chebyshev weight matrices per layer, fold the mean-pool 1/1024 into fcW1.
  - Device per graph per layer: Clenshaw recurrence
        b_k = 2L b_{k+1} - b_{k+2} + X W_k
    so every Lhat application is a dense [128,1024]^T @ [128,F] PSUM-chain
    matmul in node-layout; projections X W_k run from a feature-layout copy
    of the layer input (one PE-transpose per layer boundary). Pooling via
    matmul with a ones-vector; biases via K=1 ones-row matmuls.
"""

import os

import numpy as np

N_NODES = 65536
N_EDGES = 1048576
G = 64
NPG = 1024
IN_F = 128
HID = 64
NCLS = 10
S = 5
NCORES = 8
GPC = G // NCORES  # graphs per core

# (F_in, F_out) per ChebConv layer
LAYERS = [(128, 32), (32, 64), (64, 64)]

FP16 = True
LAST = None  # BassKernelResults of the most recent run (for test harness)
_CACHE = {}


def _build_bass(reps=1, fp16=True):
    from contextlib import ExitStack

    import concourse.bass as bass  # noqa: F401
    import concourse.tile as tile
    from concourse import bacc, mybir
    from concourse.masks import make_identity

    f32 = mybir.dt.float32
    dt = mybir.dt.float16 if fp16 else f32
    Act = mybir.ActivationFunctionType
    Alu = mybir.AluOpType

    nc = bacc.Bacc(
        "TRN2",
        target_bir_lowering=False,
        debug=False,
        enable_asserts=False,
        num_devices=NCORES,
    )

    lt_d = nc.dram_tensor("lt2", [GPC, 128, 8 * 1024], dt, kind="ExternalInput").ap()
    xt_d = nc.dram_tensor("xt", [GPC, 128, 1024], dt, kind="ExternalInput").ap()
    wall_d = [
        nc.dram_tensor(f"wall{i}", [fi, 5 * fo], dt, kind="ExternalInput").ap()
        for i, (fi, fo) in enumerate(LAYERS)
    ]
    brow_d = [
        nc.dram_tensor(f"brow{i}", [1, fo], dt, kind="ExternalInput").ap()
        for i, (fi, fo) in enumerate(LAYERS)
    ]
    fcw1_d = nc.dram_tensor("fcw1", [HID, NCLS], dt, kind="ExternalInput").ap()
    fcb1_d = nc.dram_tensor("fcb1", [1, NCLS], dt, kind="ExternalInput").ap()
    fcw2_d = nc.dram_tensor("fcw2", [NCLS, NCLS], dt, kind="ExternalInput").ap()
    fcb2_d = nc.dram_tensor("fcb2", [1, NCLS], dt, kind="ExternalInput").ap()
    out_d = nc.dram_tensor("out", [GPC, NCLS], f32, kind="ExternalOutput").ap()

    with tile.TileContext(nc) as tc, ExitStack() as ctx:
        consts = ctx.enter_context(tc.tile_pool(name="consts", bufs=1))
        ltp = ctx.enter_context(tc.tile_pool(name="lt", bufs=4))
        xtp = ctx.enter_context(tc.tile_pool(name="xtp", bufs=4))
        xfp = ctx.enter_context(tc.tile_pool(name="xfp", bufs=4))
        up = ctx.enter_context(tc.tile_pool(name="up", bufs=4))
        bp = ctx.enter_context(tc.tile_pool(name="bp", bufs=4))
        hp = ctx.enter_context(tc.tile_pool(name="hp", bufs=4))
        gp = ctx.enter_context(tc.tile_pool(name="gp", bufs=1))
        psU = None  # merged into psS below
        psA = ctx.enter_context(tc.tile_pool(name="psA", bufs=5, space="PSUM"))
        psT = ctx.enter_context(tc.tile_pool(name="psT", bufs=1, space="PSUM"))
        psS = ctx.enter_context(tc.tile_pool(name="psS", bufs=2, space="PSUM"))

        ident = consts.tile([128, 128], dt)
        make_identity(nc, ident[:])
        onesrow = consts.tile([1, 128], dt)
        nc.vector.memset(onesrow[:], 1.0)
        onescol = consts.tile([128, 1], dt)
        nc.vector.memset(onescol[:], 1.0)

        walls, brows = [], []
        for i, (fi, fo) in enumerate(LAYERS):
            wt = consts.tile([fi, 5 * fo], dt, tag=f"wall{i}")
            nc.sync.dma_start(out=wt[:], in_=wall_d[i])
            walls.append(wt)
            bt = consts.tile([1, fo], dt, tag=f"brow{i}")
            nc.sync.dma_start(out=bt[:], in_=brow_d[i])
            brows.append(bt)
        fcw1 = consts.tile([HID, NCLS], dt, tag="fcw1")
        nc.sync.dma_start(out=fcw1[:], in_=fcw1_d)
        fcb1 = consts.tile([1, NCLS], dt, tag="fcb1")
        nc.sync.dma_start(out=fcb1[:], in_=fcb1_d)
        fcw2 = consts.tile([NCLS, NCLS], dt, tag="fcw2")
        nc.sync.dma_start(out=fcw2[:], in_=fcw2_d)
        fcb2 = consts.tile([1, NCLS], dt, tag="fcb2")
        nc.sync.dma_start(out=fcb2[:], in_=fcb2_d)

        gbuf = gp.tile([HID, GPC], dt)

        def graph_prog(g):
            lt = ltp.tile([128, 8 * 1024], dt, tag="lt")
            nc.sync.dma_start(out=lt[:], in_=lt_d[g])
            xf = xtp.tile([128, 1024], dt, tag="xf")
            nc.sync.dma_start(out=xf[:], in_=xt_d[g])
            yield

            X = xf
            for li, (fi, fo) in enumerate(LAYERS):
                wall, brow = walls[li], brows[li]
                # --- only U4 = X @ W4 is materialized; the other projections
                # accumulate straight into the Lhat PSUM groups below
                U4 = up.tile([128, 8 * fo], dt, tag=f"U{li}")
                for m in range(8):
                    pu = psS.tile([128, fo], f32, tag="small")
                    nc.tensor.matmul(
                        pu[:],
                        lhsT=X[:fi, m * 128 : (m + 1) * 128],
                        rhs=wall[:, 4 * fo : 5 * fo],
                        start=True,
                        stop=True,
                    )
                    nc.scalar.copy(U4[:, m * fo : (m + 1) * fo], pu[:])
                yield

                def app(rhs_fn, wblk, with_bias=False):
                    # pa[:, m] = X_m @ W_wblk (+ bias) + sum_k 2L[k,m]^T rhs(k)
                    pa = psA.tile([128, 8 * fo], f32, tag="pa")
                    for m in range(8):
                        o = pa[:, m * fo : (m + 1) * fo]
                        nc.tensor.matmul(
                            o,
                            lhsT=X[:fi, m * 128 : (m + 1) * 128],
                            rhs=wall[:, wblk * fo : (wblk + 1) * fo],
                            start=True,
                            stop=False,
                        )
                        if with_bias:
                            nc.tensor.matmul(
                                o, lhsT=onesrow[:1, :128], rhs=brow[:], start=False, stop=False
                            )
                        for k in range(8):
                            nc.tensor.matmul(
                                o,
                                lhsT=lt[:, k * 1024 + m * 128 : k * 1024 + (m + 1) * 128],
                                rhs=rhs_fn(k),
                                start=False,
                                stop=(k == 7),
                            )
                    return pa

                b3 = bp.tile([128, 8 * fo], dt, tag="b3")
                b2 = bp.tile([128, 8 * fo], dt, tag="b2")
                b1 = bp.tile([128, 8 * fo], dt, tag="b1")
                hpre = hp.tile([128, 8 * fo], dt, tag="hpre")
                hout = hp.tile([128, 8 * fo], dt, tag="hout")

                # b3 = 2L b4 + U3           (b4 == U4)
                pa = app(lambda k: U4[:, k * fo : (k + 1) * fo], wblk=3)
                nc.vector.tensor_copy(b3[:], pa[:])
                yield
                # b2 = 2L b3 + U2 - b4
                pa = app(lambda k: b3[:, k * fo : (k + 1) * fo], wblk=2)
                nc.vector.tensor_sub(b2[:], pa[:], U4[:])
                yield
                # b1 = 2L b2 + U1 - b3
                pa = app(lambda k: b2[:, k * fo : (k + 1) * fo], wblk=1)
                nc.vector.tensor_sub(b1[:], pa[:], b3[:])
                yield
                # h = relu(0.5*(2L b1 + 2 U0 + 2 bias) - b2)   (W0/b doubled on host)
                pa = app(lambda k: b1[:, k * fo : (k + 1) * fo], wblk=0, with_bias=True)
                nc.vector.scalar_tensor_tensor(
                    hpre[:], pa[:], 0.5, b2[:], op0=Alu.mult, op1=Alu.subtract
                )
                nc.scalar.activation(hout[:], hpre[:], Act.Relu)
                yield

                if li < 2:
                    # transpose h -> feature-layout input of next layer
                    pt = psT.tile([fo, 1024], dt, tag="pt")
                    for c in range(8):
                        nc.tensor.transpose(
                            pt[:, c * 128 : (c + 1) * 128],
                            hout[:, c * fo : (c + 1) * fo],
                            ident[:],
                        )
                    Xn = xfp.tile([fo, 1024], dt, tag=f"X{li + 1}")
                    nc.scalar.copy(Xn[:], pt[:])
                    X = Xn
                else:
                    # global mean pool (1/1024 folded into fcw1 on host)
                    pp = psS.tile([HID, 1], f32, tag="small")
                    for k in range(8):
                        nc.tensor.matmul(
                            pp[:],
                            lhsT=hout[:, k * HID : (k + 1) * HID],
                            rhs=onescol[:],
                            start=(k == 0),
                            stop=(k == 7),
                        )
                    nc.scalar.copy(gbuf[:, g : g + 1], pp[:])
            yield

        order = [gg for _ in range(reps) for gg in range(GPC)]
        for p in range(0, len(order), 4):
            gens = [graph_prog(gg) for gg in order[p : p + 4]]
            alive = list(gens)
            while alive:
                for gen in list(alive):
                    try:
                        next(gen)
                    except StopIteration:
                        alive.remove(gen)

        # --- MLP head over all 8 graphs at once
        pm = psS.tile([GPC, NCLS], f32, tag="small")
        nc.tensor.matmul(pm[:], lhsT=gbuf[:], rhs=fcw1[:], start=True, stop=False)
        nc.tensor.matmul(
            pm[:], lhsT=onesrow[:1, :GPC], rhs=fcb1[:], start=False, stop=True
        )
        h1 = hp.tile([GPC, NCLS], dt, tag="mlph1")
        nc.scalar.activation(h1[:], pm[:], Act.Relu)

        ptm = psS.tile([NCLS, GPC], dt, tag="small")
        nc.tensor.transpose(ptm[:], h1[:], ident[:GPC, :GPC])
        h1t = hp.tile([NCLS, GPC], dt, tag="mlph1t")
        nc.scalar.copy(h1t[:], ptm[:])

        pf = psS.tile([GPC, NCLS], f32, tag="small")
        nc.tensor.matmul(pf[:], lhsT=h1t[:], rhs=fcw2[:], start=True, stop=False)
        nc.tensor.matmul(
            pf[:], lhsT=onesrow[:1, :GPC], rhs=fcb2[:], start=False, stop=True
        )
        ob = hp.tile([GPC, NCLS], f32, tag="ob")
        nc.vector.tensor_copy(ob[:], pf[:])
        nc.sync.dma_start(out=out_d, in_=ob[:])

    nc.compile()
    return nc


def _prep_inputs(x, edge_index, batch, lambda_max, W1, b1, W2, b2, W3, b3, fcW1, fcb1, fcW2, fcb2):
    x = np.asarray(x, np.float32)
    edge_index = np.asarray(edge_index, np.int64)
    batch = np.asarray(batch, np.int64)
    lambda_max = np.asarray(lambda_max, np.float32)

    src, dst = edge_index[0], edge_index[1]
    # the decomposition below requires block-aligned graphs; guaranteed by
    # the reference input generator
    assert np.array_equal(batch, np.arange(N_NODES) // NPG)
    assert ((src // NPG) == (dst // NPG)).all()

    mask = src != dst
    deg = np.bincount(src[mask], minlength=N_NODES).astype(np.float32)
    dis = np.where(deg > 0, 1.0 / np.sqrt(np.maximum(deg, 1.0)), 0.0).astype(np.float32)
    lam_e = lambda_max[batch[src]]
    w = np.where(mask, -2.0 * dis[src] * dis[dst] / lam_e, 0.0).astype(np.float32)
    diag = (2.0 / lambda_max[batch] - 1.0).astype(np.float32)

    ge = src // NPG
    sl = src % NPG
    dl = dst % NPG
    flat = (ge * NPG + sl) * NPG + dl
    M2 = np.bincount(flat, weights=(2.0 * w).astype(np.float64), minlength=G * NPG * NPG)
    M2 = M2.astype(np.float32).reshape(G, NPG, NPG)
    M2[:, np.arange(NPG), np.arange(NPG)] += 2.0 * diag.reshape(G, NPG)

    walls = []
    brows = []
    for Wl, bl in ((W1, b1), (W2, b2), (W3, b3)):
        Wl = np.asarray(Wl, np.float32)
        bl = np.asarray(bl, np.float32)
        fo = Wl.shape[2]
        blocks = [Wl[k].copy() for k in range(S)]
        blocks[0] = blocks[0] * 2.0
        walls.append(np.concatenate(blocks, axis=1))
        brows.append((2.0 * bl).reshape(1, fo).astype(np.float32))

    cnt = np.bincount(batch, minlength=G)
    assert (cnt == NPG).all()
    fcw1s = (np.asarray(fcW1, np.float32) / float(NPG)).astype(np.float32)

    ddt = np.float16 if FP16 else np.float32
    in_maps = []
    for c in range(NCORES):
        gs = slice(c * GPC, (c + 1) * GPC)
        lt2 = (
            M2[gs]
            .reshape(GPC, 8, 128, NPG)
            .transpose(0, 2, 1, 3)
            .reshape(GPC, 128, 8 * NPG)
            .copy()
        )
        xt = (
            x[c * GPC * NPG : (c + 1) * GPC * NPG]
            .reshape(GPC, NPG, IN_F)
            .transpose(0, 2, 1)
            .copy()
        )
        m = {
            "lt2": lt2.astype(ddt),
            "xt": xt.astype(ddt),
            "fcw1": fcw1s.astype(ddt),
            "fcb1": np.asarray(fcb1, np.float32).reshape(1, NCLS).astype(ddt),
            "fcw2": np.asarray(fcW2, np.float32).astype(ddt),
            "fcb2": np.asarray(fcb2, np.float32).reshape(1, NCLS).astype(ddt),
        }
        for i in range(3):
            m[f"wall{i}"] = walls[i].astype(ddt)
            m[f"brow{i}"] = brows[i].astype(ddt)
        in_maps.append(m)
    return in_maps


def kernel(**inputs) -> np.ndarray:
    global LAST
    from concourse.bass_utils import run_bass_kernel_spmd

    in_maps = _prep_inputs(**inputs)
    if "nc" not in _CACHE:
        _CACHE["nc"] = _build_bass(fp16=FP16)
    nc = _CACHE["nc"]
    res = run_bass_kernel_spmd(
        nc,
        in_maps,
        list(range(NCORES)),
        trace=bool(os.environ.get("KERNEL_TRACE")),
    )
    LAST = res
    out = np.concatenate([res.results[c]["out"] for c in range(NCORES)], axis=0)
    return out.astype(np.float32)



# revision 7
# speedup vs baseline: 1.9985x; 1.9985x over previous
"""ChebNet (3x ChebConv S=5 + global mean pool + 2-layer MLP) on 8 trn2 cores.

Strategy (graph-level data parallelism):
  - 64 independent graphs (1024 nodes each, edges strictly intra-graph).
    Core c owns graphs [8c, 8c+8).
  - Host prep: build per-graph dense scaled Laplacian  M2[src, dst] =
    2*Lhat[dst, src]  (edge weights + diagonal folded in, duplicate edges
    accumulated), transpose x per graph to feature-layout, concat the 5
    Chebyshev weight matrices per layer, fold the mean-pool 1/1024 into fcW1.
  - Device per graph per layer: Clenshaw recurrence
        b_k = 2L b_{k+1} - b_{k+2} + X W_k
    so every Lhat application is a dense [128,1024]^T @ [128,F] PSUM-chain
    matmul in node-layout; projections X W_k run from a feature-layout copy
    of the layer input (one PE-transpose per layer boundary). Pooling via
    matmul with a ones-vector; biases via K=1 ones-row matmuls.
"""

import os

import numpy as np

N_NODES = 65536
N_EDGES = 1048576
G = 64
NPG = 1024
IN_F = 128
HID = 64
NCLS = 10
S = 5
NCORES = 8
GPC = G // NCORES  # graphs per core

# (F_in, F_out) per ChebConv layer
LAYERS = [(128, 32), (32, 64), (64, 64)]

FP16 = True
FP8_L = True  # store the dense Laplacian blocks in fp8e4 (stationary operand)
LAST = None  # BassKernelResults of the most recent run (for test harness)
_CACHE = {}


def _build_bass(reps=1, fp16=True):
    from contextlib import ExitStack

    import concourse.bass as bass  # noqa: F401
    import concourse.tile as tile
    from concourse import bacc, mybir
    from concourse.masks import make_identity

    f32 = mybir.dt.float32
    dt = mybir.dt.float16 if fp16 else f32
    dtL = mybir.dt.float8e4 if FP8_L else dt
    Act = mybir.ActivationFunctionType
    Alu = mybir.AluOpType

    nc = bacc.Bacc(
        "TRN2",
        target_bir_lowering=False,
        debug=False,
        enable_asserts=False,
        num_devices=NCORES,
    )

    lt_d = nc.dram_tensor("lt2", [GPC, 128, 8 * 1024], dtL, kind="ExternalInput").ap()
    xt_d = nc.dram_tensor("xt", [GPC, 128, 1024], dt, kind="ExternalInput").ap()
    wall_d = [
        nc.dram_tensor(f"wall{i}", [fi, 5 * fo], dt, kind="ExternalInput").ap()
        for i, (fi, fo) in enumerate(LAYERS)
    ]
    brow_d = [
        nc.dram_tensor(f"brow{i}", [1, fo], dt, kind="ExternalInput").ap()
        for i, (fi, fo) in enumerate(LAYERS)
    ]
    fcw1_d = nc.dram_tensor("fcw1", [HID, NCLS], dt, kind="ExternalInput").ap()
    fcb1_d = nc.dram_tensor("fcb1", [1, NCLS], dt, kind="ExternalInput").ap()
    fcw2_d = nc.dram_tensor("fcw2", [NCLS, NCLS], dt, kind="ExternalInput").ap()
    fcb2_d = nc.dram_tensor("fcb2", [1, NCLS], dt, kind="ExternalInput").ap()
    out_d = nc.dram_tensor("out", [GPC, NCLS], f32, kind="ExternalOutput").ap()

    with tile.TileContext(nc) as tc, ExitStack() as ctx:
        consts = ctx.enter_context(tc.tile_pool(name="consts", bufs=1))
        ltp = ctx.enter_context(tc.tile_pool(name="lt", bufs=4))
        xtp = ctx.enter_context(tc.tile_pool(name="xtp", bufs=4))
        xfp = ctx.enter_context(tc.tile_pool(name="xfp", bufs=4))
        up = ctx.enter_context(tc.tile_pool(name="up", bufs=4))
        bp = ctx.enter_context(tc.tile_pool(name="bp", bufs=4))
        hp = ctx.enter_context(tc.tile_pool(name="hp", bufs=4))
        gp = ctx.enter_context(tc.tile_pool(name="gp", bufs=1))
        psU = None  # merged into psS below
        psA = ctx.enter_context(tc.tile_pool(name="psA", bufs=5, space="PSUM"))
        psT = ctx.enter_context(tc.tile_pool(name="psT", bufs=1, space="PSUM"))
        psS = ctx.enter_context(tc.tile_pool(name="psS", bufs=2, space="PSUM"))

        ident = consts.tile([128, 128], dt)
        make_identity(nc, ident[:])
        onesrow = consts.tile([1, 128], dt)
        nc.vector.memset(onesrow[:], 1.0)
        onescol = consts.tile([128, 1], dt)
        nc.vector.memset(onescol[:], 1.0)

        walls, brows = [], []
        for i, (fi, fo) in enumerate(LAYERS):
            wt = consts.tile([fi, 5 * fo], dt, tag=f"wall{i}")
            nc.sync.dma_start(out=wt[:], in_=wall_d[i])
            walls.append(wt)
            bt = consts.tile([1, fo], dt, tag=f"brow{i}")
            nc.sync.dma_start(out=bt[:], in_=brow_d[i])
            brows.append(bt)
        fcw1 = consts.tile([HID, NCLS], dt, tag="fcw1")
        nc.sync.dma_start(out=fcw1[:], in_=fcw1_d)
        fcb1 = consts.tile([1, NCLS], dt, tag="fcb1")
        nc.sync.dma_start(out=fcb1[:], in_=fcb1_d)
        fcw2 = consts.tile([NCLS, NCLS], dt, tag="fcw2")
        nc.sync.dma_start(out=fcw2[:], in_=fcw2_d)
        fcb2 = consts.tile([1, NCLS], dt, tag="fcb2")
        nc.sync.dma_start(out=fcb2[:], in_=fcb2_d)

        gbuf = gp.tile([HID, GPC], dt)

        def graph_prog(g):
            lt = ltp.tile([128, 8 * 1024], dtL, tag="lt")
            nc.sync.dma_start(out=lt[:], in_=lt_d[g])
            xf = xtp.tile([128, 1024], dt, tag="xf")
            nc.sync.dma_start(out=xf[:], in_=xt_d[g])
            yield

            X = xf
            for li, (fi, fo) in enumerate(LAYERS):
                wall, brow = walls[li], brows[li]
                # --- only U4 = X @ W4 is materialized; the other projections
                # accumulate straight into the Lhat PSUM groups below
                U4 = up.tile([128, 8 * fo], dt, tag=f"U{li}")
                for m in range(8):
                    pu = psS.tile([128, fo], f32, tag="small")
                    nc.tensor.matmul(
                        pu[:],
                        lhsT=X[:fi, m * 128 : (m + 1) * 128],
                        rhs=wall[:, 4 * fo : 5 * fo],
                        start=True,
                        stop=True,
                    )
                    nc.scalar.copy(U4[:, m * fo : (m + 1) * fo], pu[:])
                yield

                def app(rhs_fn, wblk, with_bias=False):
                    # pa[:, m] = X_m @ W_wblk (+ bias) + sum_k 2L[k,m]^T rhs(k)
                    pa = psA.tile([128, 8 * fo], f32, tag="pa")
                    for m in range(8):
                        o = pa[:, m * fo : (m + 1) * fo]
                        nc.tensor.matmul(
                            o,
                            lhsT=X[:fi, m * 128 : (m + 1) * 128],
                            rhs=wall[:, wblk * fo : (wblk + 1) * fo],
                            start=True,
                            stop=False,
                        )
                        if with_bias:
                            nc.tensor.matmul(
                                o, lhsT=onesrow[:1, :128], rhs=brow[:], start=False, stop=False
                            )
                        for k in range(8):
                            nc.tensor.matmul(
                                o,
                                lhsT=lt[:, k * 1024 + m * 128 : k * 1024 + (m + 1) * 128],
                                rhs=rhs_fn(k),
                                start=False,
                                stop=(k == 7),
                            )
                    return pa

                b3 = bp.tile([128, 8 * fo], dt, tag="b3")
                b2 = bp.tile([128, 8 * fo], dt, tag="b2")
                b1 = bp.tile([128, 8 * fo], dt, tag="b1")
                hpre = hp.tile([128, 8 * fo], dt, tag="hpre")
                hout = hp.tile([128, 8 * fo], dt, tag="hout")

                # b3 = 2L b4 + U3           (b4 == U4)
                pa = app(lambda k: U4[:, k * fo : (k + 1) * fo], wblk=3)
                nc.vector.tensor_copy(b3[:], pa[:])
                yield
                # b2 = 2L b3 + U2 - b4
                pa = app(lambda k: b3[:, k * fo : (k + 1) * fo], wblk=2)
                nc.vector.tensor_sub(b2[:], pa[:], U4[:])
                yield
                # b1 = 2L b2 + U1 - b3
                pa = app(lambda k: b2[:, k * fo : (k + 1) * fo], wblk=1)
                nc.vector.tensor_sub(b1[:], pa[:], b3[:])
                yield
                # h = relu(0.5*(2L b1 + 2 U0 + 2 bias) - b2)   (W0/b doubled on host)
                pa = app(lambda k: b1[:, k * fo : (k + 1) * fo], wblk=0, with_bias=True)
                nc.vector.scalar_tensor_tensor(
                    hpre[:], pa[:], 0.5, b2[:], op0=Alu.mult, op1=Alu.subtract
                )
                nc.scalar.activation(hout[:], hpre[:], Act.Relu)
                yield

                if li < 2:
                    # transpose h -> feature-layout input of next layer
                    pt = psT.tile([fo, 1024], dt, tag="pt")
                    for c in range(8):
                        nc.tensor.transpose(
                            pt[:, c * 128 : (c + 1) * 128],
                            hout[:, c * fo : (c + 1) * fo],
                            ident[:],
                        )
                    Xn = xfp.tile([fo, 1024], dt, tag=f"X{li + 1}")
                    nc.scalar.copy(Xn[:], pt[:])
                    X = Xn
                else:
                    # global mean pool (1/1024 folded into fcw1 on host)
                    pp = psS.tile([HID, 1], f32, tag="small")
                    for k in range(8):
                        nc.tensor.matmul(
                            pp[:],
                            lhsT=hout[:, k * HID : (k + 1) * HID],
                            rhs=onescol[:],
                            start=(k == 0),
                            stop=(k == 7),
                        )
                    nc.scalar.copy(gbuf[:, g : g + 1], pp[:])
            yield

        order = [gg for _ in range(reps) for gg in range(GPC)]
        for p in range(0, len(order), 4):
            gens = [graph_prog(gg) for gg in order[p : p + 4]]
            alive = list(gens)
            while alive:
                for gen in list(alive):
                    try:
                        next(gen)
                    except StopIteration:
                        alive.remove(gen)

        # --- MLP head over all 8 graphs at once
        pm = psS.tile([GPC, NCLS], f32, tag="small")
        nc.tensor.matmul(pm[:], lhsT=gbuf[:], rhs=fcw1[:], start=True, stop=False)
        nc.tensor.matmul(
            pm[:], lhsT=onesrow[:1, :GPC], rhs=fcb1[:], start=False, stop=True
        )
        h1 = hp.tile([GPC, NCLS], dt, tag="mlph1")
        nc.scalar.activation(h1[:], pm[:], Act.Relu)

        ptm = psS.tile([NCLS, GPC], dt, tag="small")
        nc.tensor.transpose(ptm[:], h1[:], ident[:GPC, :GPC])
        h1t = hp.tile([NCLS, GPC], dt, tag="mlph1t")
        nc.scalar.copy(h1t[:], ptm[:])

        pf = psS.tile([GPC, NCLS], f32, tag="small")
        nc.tensor.matmul(pf[:], lhsT=h1t[:], rhs=fcw2[:], start=True, stop=False)
        nc.tensor.matmul(
            pf[:], lhsT=onesrow[:1, :GPC], rhs=fcb2[:], start=False, stop=True
        )
        ob = hp.tile([GPC, NCLS], f32, tag="ob")
        nc.vector.tensor_copy(ob[:], pf[:])
        nc.sync.dma_start(out=out_d, in_=ob[:])

    nc.compile()
    return nc


def _prep_inputs(x, edge_index, batch, lambda_max, W1, b1, W2, b2, W3, b3, fcW1, fcb1, fcW2, fcb2):
    x = np.asarray(x, np.float32)
    edge_index = np.asarray(edge_index, np.int64)
    batch = np.asarray(batch, np.int64)
    lambda_max = np.asarray(lambda_max, np.float32)

    src, dst = edge_index[0], edge_index[1]
    # the decomposition below requires block-aligned graphs; guaranteed by
    # the reference input generator
    assert np.array_equal(batch, np.arange(N_NODES) // NPG)
    assert ((src // NPG) == (dst // NPG)).all()

    mask = src != dst
    deg = np.bincount(src[mask], minlength=N_NODES).astype(np.float32)
    dis = np.where(deg > 0, 1.0 / np.sqrt(np.maximum(deg, 1.0)), 0.0).astype(np.float32)
    lam_e = lambda_max[batch[src]]
    w = np.where(mask, -2.0 * dis[src] * dis[dst] / lam_e, 0.0).astype(np.float32)
    diag = (2.0 / lambda_max[batch] - 1.0).astype(np.float32)

    ge = src // NPG
    sl = src % NPG
    dl = dst % NPG
    flat = (ge * NPG + sl) * NPG + dl
    M2 = np.bincount(flat, weights=(2.0 * w).astype(np.float64), minlength=G * NPG * NPG)
    M2 = M2.astype(np.float32).reshape(G, NPG, NPG)
    M2[:, np.arange(NPG), np.arange(NPG)] += 2.0 * diag.reshape(G, NPG)

    walls = []
    brows = []
    for Wl, bl in ((W1, b1), (W2, b2), (W3, b3)):
        Wl = np.asarray(Wl, np.float32)
        bl = np.asarray(bl, np.float32)
        fo = Wl.shape[2]
        blocks = [Wl[k].copy() for k in range(S)]
        blocks[0] = blocks[0] * 2.0
        walls.append(np.concatenate(blocks, axis=1))
        brows.append((2.0 * bl).reshape(1, fo).astype(np.float32))

    cnt = np.bincount(batch, minlength=G)
    assert (cnt == NPG).all()
    fcw1s = (np.asarray(fcW1, np.float32) / float(NPG)).astype(np.float32)

    ddt = np.float16 if FP16 else np.float32
    if FP8_L:
        import ml_dtypes

        ldt = ml_dtypes.float8_e4m3
    else:
        ldt = ddt
    in_maps = []
    for c in range(NCORES):
        gs = slice(c * GPC, (c + 1) * GPC)
        lt2 = (
            M2[gs]
            .reshape(GPC, 8, 128, NPG)
            .transpose(0, 2, 1, 3)
            .reshape(GPC, 128, 8 * NPG)
            .copy()
        )
        xt = (
            x[c * GPC * NPG : (c + 1) * GPC * NPG]
            .reshape(GPC, NPG, IN_F)
            .transpose(0, 2, 1)
            .copy()
        )
        m = {
            "lt2": lt2.astype(ldt),
            "xt": xt.astype(ddt),
            "fcw1": fcw1s.astype(ddt),
            "fcb1": np.asarray(fcb1, np.float32).reshape(1, NCLS).astype(ddt),
            "fcw2": np.asarray(fcW2, np.float32).astype(ddt),
            "fcb2": np.asarray(fcb2, np.float32).reshape(1, NCLS).astype(ddt),
        }
        for i in range(3):
            m[f"wall{i}"] = walls[i].astype(ddt)
            m[f"brow{i}"] = brows[i].astype(ddt)
        in_maps.append(m)
    return in_maps


def kernel(**inputs) -> np.ndarray:
    global LAST
    from concourse.bass_utils import run_bass_kernel_spmd

    in_maps = _prep_inputs(**inputs)
    if "nc" not in _CACHE:
        _CACHE["nc"] = _build_bass(fp16=FP16)
    nc = _CACHE["nc"]
    res = run_bass_kernel_spmd(
        nc,
        in_maps,
        list(range(NCORES)),
        trace=bool(os.environ.get("KERNEL_TRACE")),
    )
    LAST = res
    out = np.concatenate([res.results[c]["out"] for c in range(NCORES)], axis=0)
    return out.astype(np.float32)

